# revision 14
# baseline (speedup 1.0000x reference)
"""ClusteringLoss kernel for 8x Trainium2 NeuronCores.

Computes, for feature [8192, 512] and centroid_ids [64]:
  pd    = pairwise_distance(feature)           (torch-style, eps=1e-6)
  dc    = pd[:, centroid_ids]                  [N, K]
  facility_energy = -sum_i min_k dc[i, k]
  predictions     = argmin_k dc[i, k]
  y_fixed         = (1-mask)*predictions + constraint_vect

Only the K=64 centroid columns of pd are ever used, so the kernel computes
the [N, K] distance block directly:
  d2[i,k] = sq_i + sq_k - 2*f_i.c_k + 2*eps*(s_i - s_k) + D*eps^2
The row-constant 2*eps*s_i term (<= ~1.3e-4 absolute on d2 ~ 1e3) only
shifts all k equally, so it cannot change argmin; its effect on the energy
sum is ~1e-7 relative, far below fp32 noise. It is dropped. Everything that
varies with k (sq_k, -2*eps*s_k, D*eps^2) is kept exactly in a per-k
constant subtracted on the VectorEngine (exact fp32).

Sharding: rows are split 1024 per core (data parallel). Each core receives
its feature shard pre-transposed ([512, 1024], contraction on partitions)
plus the replicated centroid operands. The gram runs [k, i]-oriented
(centroids stationary, features moving) in float32r so the TensorEngine
streams one row per cycle, tiled into four i-quarters so the back end
pipelines behind the DMA stream: each quarter is const-adjusted during the
PSUM->SBUF copy (tensor_scalar add, per-partition scalar), transposed back
to [i, k] blocks with PE transpose-mode, then reduced min/argmin per row on
VectorE. Row-squared-norms ride a bf16 ones-matmul side path; min distances
are square-rooted and summed per partition on ScalarE. The host adds the
8x128 partials and applies the centroid mask/constraint fixup (O(K) work).
"""

import numpy as np

N, D, K = 8192, 512, 64
NCORES = 8
NL = N // NCORES          # 1024 rows per core
NBLK = NL // 128          # 8 row blocks of 128
NQ = 4                    # i-quarters of 256 (gram moving-dim tiles)
NCH = D // 128            # 4 contraction chunks of 128
EPS = 1e-6
BIG = 1024.0              # argmin tie-break offset; > K, exact in fp32

TRACE = False             # set True (e.g. from test.py) to capture an NTFF profile
LAST_EXEC_NS = None
_CACHE = {}


def _build_nc():
    import concourse.bacc as bacc
    import concourse.mybir as mybir
    import concourse.tile as tile

    f32 = mybir.dt.float32
    f32r = mybir.dt.float32r
    bf16 = mybir.dt.bfloat16
    QW = NL // NQ                                        # 256 cols per quarter

    nc = bacc.Bacc("TRN2", target_bir_lowering=False, debug=False,
                   num_devices=NCORES)
    # ftd[c][q] = feature_shard.T[c*128:(c+1)*128, q*256:(q+1)*256]
    ftd = nc.dram_tensor("ftd", [NCH, NQ, 128, QW], f32r, kind="ExternalInput")
    rhs2 = nc.dram_tensor("rhs2", [128, NCH * K], f32r, kind="ExternalInput")
    ncol = nc.dram_tensor("ncol", [64, 1], f32, kind="ExternalInput")
    eye = nc.dram_tensor("eye", [64, 64], f32, kind="ExternalInput")
    out = nc.dram_tensor("out", [128, NBLK + 1], f32, kind="ExternalOutput")

    with tile.TileContext(nc) as tc:
        with (
            tc.tile_pool(name="const", bufs=1) as cpool,
            tc.tile_pool(name="ft", bufs=NCH * NQ) as ftpool,
            tc.tile_pool(name="fsq", bufs=NCH * NQ) as fsqpool,
            tc.tile_pool(name="psg", bufs=2, space="PSUM") as psg,
            tc.tile_pool(name="psq", bufs=NQ, space="PSUM") as psq,
            tc.tile_pool(name="pse", bufs=1, space="PSUM") as pse,
            tc.tile_pool(name="ep", bufs=1) as eppool,
        ):
            # small constants first (scalar HWDGE ring) so the first gram
            # matmul is never waiting on them
            rhs_sb = cpool.tile([128, NCH * K], f32r)
            nc.scalar.dma_start(rhs_sb[:, :], rhs2[:, :])
            ncol_sb = cpool.tile([64, 1], f32)
            nc.scalar.dma_start(ncol_sb[:, :], ncol[:, :])
            eye_sb = cpool.tile([64, 64], f32)
            nc.scalar.dma_start(eye_sb[:, :], eye[:, :])
            onesb = cpool.tile([128, 1], bf16)
            nc.vector.memset(onesb[:, :], 1.0)
            iota_t = cpool.tile([128, NBLK * K], f32)
            nc.gpsimd.iota(iota_t[:, :].rearrange("p (b k) -> p b k", k=K),
                           pattern=[[0, NBLK], [1, K]], base=int(BIG),
                           channel_multiplier=0,
                           allow_small_or_imprecise_dtypes=True)

            sq_ps = pse.tile([128, NBLK], f32, tag="sq")
            m_sb = eppool.tile([128, NBLK], f32)
            msk = eppool.tile([128, NBLK * K], f32)
            sel = eppool.tile([128, NBLK * K], f32)
            out_sb = eppool.tile([128, NBLK + 1], f32)

            # DMA all quarters up front (alternating the two HWDGE paths by
            # contraction-chunk parity) -- transfers drain in issue order
            ft_t = [[None] * NQ for _ in range(NCH)]
            for q in range(NQ):
                for c in range(NCH):
                    t = ftpool.tile([128, QW], f32r, tag="ft",
                                    name=f"ft{c}_{q}")
                    eng = nc.sync if c % 2 == 0 else nc.gpsimd
                    eng.dma_start(t[:, :], ftd[c, q, :, :])
                    ft_t[c][q] = t

            nsq = 0
            for q in range(NQ):
                # g[k, i'] = 2*f_i.c_k for this quarter, contracted over c
                g_ps = psg.tile([64, QW], f32, tag="g", name=f"g_ps{q}")
                for c in range(NCH):
                    nc.tensor.matmul(g_ps[:, :],
                                     lhsT=rhs_sb[:, c * K:(c + 1) * K],
                                     rhs=ft_t[c][q][:, :],
                                     start=(c == 0), stop=(c == NCH - 1))
                # PSUM -> SBUF with the exact -const_k folded in (k is the
                # partition dim here, so it is a per-partition scalar add)
                g_sb = eppool.tile([64, QW], f32, tag="gsb", name=f"g_sb{q}")
                nc.vector.tensor_scalar_add(g_sb[:, :], g_ps[:, :],
                                            ncol_sb[:, 0:1])
                # back to [i, k] row blocks (own PSUM bank per quarter so the
                # reductions can read while later quarters are still writing)
                et_ps = psq.tile([128, 2 * K], f32, tag="et", name=f"et_ps{q}")
                for j in range(2):
                    nc.tensor.matmul(et_ps[:, j * K:(j + 1) * K],
                                     lhsT=g_sb[:, j * 128:(j + 1) * 128],
                                     rhs=eye_sb[:, :], is_transpose=True,
                                     start=(j == 0), stop=(j == 1))
                # row min/argmin for this quarter's two blocks
                e3 = et_ps[:, :].rearrange("p (b k) -> p b k", k=K)
                mq = m_sb[:, 2 * q:2 * q + 2]
                nc.vector.tensor_reduce(mq, e3, axis=mybir.AxisListType.X,
                                        op=mybir.AluOpType.max)
                nc.vector.tensor_tensor(
                    out=msk[:, q * 2 * K:(q + 1) * 2 * K].rearrange(
                        "p (b k) -> p b k", k=K),
                    in0=e3, in1=mq.broadcast_to((128, 2, K)),
                    op=mybir.AluOpType.is_ge)
                nc.vector.scalar_tensor_tensor(
                    out=sel[:, q * 2 * K:(q + 1) * 2 * K],
                    in0=msk[:, q * 2 * K:(q + 1) * 2 * K],
                    scalar=-BIG,
                    in1=iota_t[:, q * 2 * K:(q + 1) * 2 * K],
                    op0=mybir.AluOpType.mult, op1=mybir.AluOpType.add)
                nc.vector.tensor_reduce(
                    out_sb[:, 2 * q:2 * q + 2],
                    sel[:, q * 2 * K:(q + 1) * 2 * K].rearrange(
                        "p (b k) -> p b k", k=K),
                    axis=mybir.AxisListType.X, op=mybir.AluOpType.min)
                # squared-norm side path (bf16; error is row-constant for
                # argmin and ~1e-7 relative on the energy)
                for c in range(NCH):
                    sq = fsqpool.tile([128, QW], bf16, tag="fsq",
                                      name=f"fsq{c}_{q}")
                    nc.scalar.activation(sq[:, :],
                                         ft_t[c][q][:, :].bitcast(f32),
                                         mybir.ActivationFunctionType.Square)
                    for j in range(2):
                        b = 2 * q + j
                        nc.tensor.matmul(sq_ps[:, b:b + 1],
                                         lhsT=sq[:, j * 128:(j + 1) * 128],
                                         rhs=onesb[:, :],
                                         start=(nsq == 0),
                                         stop=(nsq == NCH * NBLK - 1))
                        nsq += 1

            # d2min = sq_i - max_k e, clamped at 0 (torch clamps too)
            d2m = eppool.tile([128, NBLK], f32)
            nc.vector.tensor_tensor(out=d2m[:, :], in0=sq_ps[:, :],
                                    in1=m_sb[:, :],
                                    op=mybir.AluOpType.subtract)
            nc.vector.tensor_scalar_max(d2m[:, :], d2m[:, :], 0.0)
            dmin = eppool.tile([128, NBLK], f32)
            nc.scalar.activation(dmin[:, :], d2m[:, :],
                                 mybir.ActivationFunctionType.Sqrt,
                                 accum_out=out_sb[:, NBLK:NBLK + 1])
            nc.sync.dma_start(out[:, :], out_sb[:, :])

    nc.compile()
    return nc


def _get_nc():
    if "nc" not in _CACHE:
        _CACHE["nc"] = _build_nc()
    return _CACHE["nc"]


def kernel(feature, centroid_ids):
    global LAST_EXEC_NS
    from concourse.bass_utils import run_bass_kernel_spmd

    feature = np.ascontiguousarray(np.asarray(feature, dtype=np.float32))
    ids = np.asarray(centroid_ids).astype(np.int64)
    assert feature.shape == (N, D)
    assert ids.shape == (K,)

    # Deduplicate centroids (duplicate ids produce identical distance
    # columns; jnp.argmin takes the first occurrence, so duplicates can
    # never win -- map device argmin over unique centroids back to the
    # first-occurrence original index).
    ids_u, first_idx = np.unique(ids, return_index=True)
    ku = ids_u.shape[0]
    C = feature[ids_u]                                   # [ku, D]
    sq_k = (C.astype(np.float64) ** 2).sum(1)
    s_k = C.astype(np.float64).sum(1)
    const = sq_k - 2.0 * EPS * s_k + D * EPS * EPS       # [ku]
    nconst = np.full(K, -1e9, dtype=np.float32)          # padding never wins
    nconst[:ku] = (-const).astype(np.float32)

    ct2 = np.zeros((D, K), dtype=np.float32)
    ct2[:, :ku] = 2.0 * C.T
    rhs2 = np.ascontiguousarray(
        ct2.reshape(NCH, 128, K).transpose(1, 0, 2).reshape(128, NCH * K))
    eye = np.eye(64, dtype=np.float32)

    ft = feature.T                                       # [D, N] view
    in_maps = []
    for r in range(NCORES):
        shard = np.ascontiguousarray(ft[:, r * NL:(r + 1) * NL])
        in_maps.append({
            "ftd": shard.reshape(NCH, 128, NQ, NL // NQ).transpose(0, 2, 1, 3).copy(),
            "rhs2": rhs2,
            "ncol": nconst[:, None],
            "eye": eye,
        })

    nc = _get_nc()
    res = run_bass_kernel_spmd(nc, in_maps, core_ids=list(range(NCORES)),
                               trace=TRACE)
    if TRACE:
        LAST_EXEC_NS = res.exec_time_ns

    preds = np.empty(N, dtype=np.int64)
    dtot = 0.0
    for r in range(NCORES):
        o = np.asarray(res.results[r]["out"])            # [128, NBLK+1]
        preds[r * NL:(r + 1) * NL] = o[:, 0:NBLK].T.flatten().astype(np.int64)
        dtot += float(o[:, NBLK].astype(np.float64).sum())

    facility_energy = np.float32(-dtot)
    pred_orig = first_idx[preds].astype(np.float32)      # back to original k

    mask = np.zeros(N, dtype=np.float32)
    constraint = np.zeros(N, dtype=np.float32)
    mask[ids] = 1.0                                      # last-wins, like XLA scatter on CPU
    constraint[ids] = np.arange(K, dtype=np.float32)
    y_fixed = (1.0 - mask) * pred_orig + constraint
    return facility_energy, y_fixed
